# revision 6
# baseline (speedup 1.0000x reference)
"""Multi-head causal attention (B=4, T=2048, C=1024, H=16, D=64) on 8 TRN2 cores.

Sharding: core c = (batch b = c//2, head-group g = c%2 of 8 heads).
Each core computes, for its batch and its 8 heads:
  QT = (Wq_g^T x_b^T), KT likewise          [E=512, T] (head-major rows)
  V  = x_b Wv_g                              [T, E] (natural orientation)
  per head: scores^T = K_h Q_h^T / sqrt(D)   [s, t] tiles, causal
            P~ = exp(scores^T) (masked)      denominator via ones-column in AV
            O'^T (+denom row) = [V_h | 1]^T P~
            O^T = O'^T * (1/denom)           broadcast via K=1 matmul
  y_part = O^T^T Wo_g                        [T, C]  (partial: this head group)
Host: y_b = y_part(g=0) + y_part(g=1) + bp.

All matmuls run in float32r (tf32-class, full PE rate); storage is f32 bits.
"""

import numpy as np

import concourse.bacc as bacc
import concourse.mybir as mybir
import concourse.tile as tile
from concourse.bass_utils import run_bass_kernel_spmd

B, T, C, H, D = 4, 2048, 1024, 16, 64
NH = 8                 # heads per core
E = NH * D             # 512 per-core head width
P = 128
KO = C // P            # 8 contraction chunks for QKV proj
ET = E // P            # 4 e-tiles
SW = 512               # stage-2 t-slab width
NSLAB = T // SW        # 4
TSL = 1024             # attention t-slab width (2 psum banks)
NSC = T // P           # 16 s-chunks
F32 = mybir.dt.float32
F32R = mybir.dt.float32r
BF16 = mybir.dt.bfloat16
EXP = mybir.ActivationFunctionType.Exp
GE = mybir.AluOpType.is_ge
MUL = mybir.AluOpType.mult
SCALE = float(D) ** -0.5

_CACHE: dict = {}


def _build():
    nc = bacc.Bacc("TRN2", target_bir_lowering=False, debug=False)
    xt_d = nc.dram_tensor("xt", [C, T], F32R, kind="ExternalInput")
    wq_d = nc.dram_tensor("wq", [C, E], F32R, kind="ExternalInput")
    wk_d = nc.dram_tensor("wk", [C, E], F32R, kind="ExternalInput")
    wv_d = nc.dram_tensor("wv", [C, E], F32R, kind="ExternalInput")
    wo_d = nc.dram_tensor("wo", [E, C], F32R, kind="ExternalInput")
    y_d = nc.dram_tensor("y", [T, C], F32, kind="ExternalOutput")

    xt_v = xt_d.ap().rearrange("(ko p) t -> p ko t", p=P)
    wq_v = wq_d.ap().rearrange("(ko p) e -> p ko e", p=P)
    wk_v = wk_d.ap().rearrange("(ko p) e -> p ko e", p=P)
    wv_v = wv_d.ap().rearrange("(ko p) e -> p ko e", p=P)
    wo_v = wo_d.ap().rearrange("(ko p) j -> p ko j", p=P)
    y_v = y_d.ap()

    with tile.TileContext(nc) as tc:
        with (
            tc.tile_pool(name="qkv", bufs=1) as qkv_pool,
            tc.tile_pool(name="vsb", bufs=1) as v_pool,
            tc.tile_pool(name="otp", bufs=1) as ot_pool,
        ):
            QT = qkv_pool.tile([P, ET, T], BF16)
            KT = qkv_pool.tile([P, ET, T], BF16)
            Vsb = v_pool.tile([P, NSC, NH, D + 1], F32R)
            OT = ot_pool.tile([P, ET, T], F32R)
            ones_t = v_pool.tile([P, 1], F32)
            nc.gpsimd.memset(ones_t[:], 1.0)
            nc.vector.tensor_copy(
                Vsb[:, :, :, D].rearrange("p a b -> p (a b)"),
                ones_t[:, 0:1].to_broadcast((P, NSC * NH)))
            ones_s = Vsb[0:1, 0:NH, :, D]  # [1, 8, 8] = 64 ones (strided)

            # ---- stage 1+2: load inputs, QKV projections ----
            with (
                tc.tile_pool(name="wgt", bufs=1) as w_pool,
                tc.tile_pool(name="xsl", bufs=2) as x_pool,
                tc.tile_pool(name="ps2", bufs=8, space="PSUM") as ps2,
            ):
                wq_s = w_pool.tile([P, KO, E], F32R)
                wk_s = w_pool.tile([P, KO, E], F32R)
                wv_s = w_pool.tile([P, KO, E], F32R)
                nc.sync.dma_start(wq_s[:], wq_v)
                nc.sync.dma_start(wk_s[:], wk_v)
                nc.sync.dma_start(wv_s[:], wv_v)

                for sl in range(NSLAB):
                    xs = x_pool.tile([P, KO, SW], F32R, tag="xs")
                    nc.sync.dma_start(xs[:], xt_v[:, :, sl * SW:(sl + 1) * SW])
                    for et in range(ET):
                        pq = ps2.tile([P, SW], F32, tag="p2")
                        for ko in range(KO):
                            nc.tensor.matmul(
                                pq[:], lhsT=wq_s[:, ko, et * P:(et + 1) * P],
                                rhs=xs[:, ko, :],
                                start=(ko == 0), stop=(ko == KO - 1))
                        nc.scalar.copy(QT[:, et, sl * SW:(sl + 1) * SW], pq[:])
                        pk = ps2.tile([P, SW], F32, tag="p2")
                        for ko in range(KO):
                            nc.tensor.matmul(
                                pk[:], lhsT=wk_s[:, ko, et * P:(et + 1) * P],
                                rhs=xs[:, ko, :],
                                start=(ko == 0), stop=(ko == KO - 1))
                        nc.vector.tensor_copy(KT[:, et, sl * SW:(sl + 1) * SW], pk[:])
                    for si in range(SW // P):
                        pv = ps2.tile([P, E], F32, tag="p2")
                        for ko in range(KO):
                            nc.tensor.matmul(
                                pv[:], lhsT=xs[:, ko, si * P:(si + 1) * P],
                                rhs=wv_s[:, ko, :],
                                start=(ko == 0), stop=(ko == KO - 1))
                        st = sl * (SW // P) + si
                        nc.vector.tensor_copy(
                            Vsb[:, st, :, 0:D],
                            pv[:].rearrange("p (h d) -> p h d", d=D))

            # ---- stage 3: attention per head ----
            with (
                tc.tile_pool(name="ptl", bufs=3) as pt_pool,
                tc.tile_pool(name="rcp", bufs=2) as r_pool,
                tc.tile_pool(name="osb", bufs=2) as o_pool,
                tc.tile_pool(name="psw", bufs=2, space="PSUM") as ps_w,
                tc.tile_pool(name="pso", bufs=2, space="PSUM") as ps_o,
            ):
                for h in range(NH):
                    eth = h // 2
                    po = (h % 2) * D
                    for tt2 in range(T // TSL):
                        t0 = tt2 * TSL
                        n_sc = (tt2 + 1) * (TSL // P) // 2 + (tt2) * 0
                        n_sc = ((t0 + TSL) // P)  # causal: s < t0+TSL
                        last_lo = min(n_sc - 1, (t0 + SW) // P - 1)
                        p_o = ps_o.tile([P, TSL], F32, tag="po")
                        for sc in range(n_sc):
                            dlt = sc * P - t0
                            lo_valid = dlt < SW
                            lo = 0 if lo_valid else SW
                            p_w = ps_w.tile([P, TSL], F32, tag="pw")
                            kk = KT[po:po + D, eth, sc * P:(sc + 1) * P]
                            if lo_valid:
                                nc.tensor.matmul(
                                    p_w[:, 0:SW], lhsT=kk,
                                    rhs=QT[po:po + D, eth, t0:t0 + SW],
                                    start=True, stop=True)
                            nc.tensor.matmul(
                                p_w[:, SW:TSL], lhsT=kk,
                                rhs=QT[po:po + D, eth, t0 + SW:t0 + TSL],
                                start=True, stop=True)
                            p_t = pt_pool.tile([P, TSL], F32R, tag="pt")
                            nc.scalar.activation(
                                p_t[:, lo:TSL], p_w[:, lo:TSL], EXP, scale=SCALE)
                            if dlt >= 0:
                                w_hi = min(dlt + P, TSL)
                                if w_hi > lo:
                                    nc.gpsimd.affine_select(
                                        out=p_t[:, lo:w_hi], in_=p_t[:, lo:w_hi],
                                        pattern=[[1, w_hi - lo]], compare_op=GE,
                                        fill=0.0, base=lo - dlt,
                                        channel_multiplier=-1)
                            vv = Vsb[:, sc, h, :]
                            if lo_valid:
                                nc.tensor.matmul(
                                    p_o[0:D + 1, 0:SW], lhsT=vv, rhs=p_t[:, 0:SW],
                                    start=(sc == 0), stop=(sc == last_lo))
                            nc.tensor.matmul(
                                p_o[0:D + 1, SW:TSL], lhsT=vv, rhs=p_t[:, SW:TSL],
                                start=(sc == 0), stop=(sc == n_sc - 1))
                        rcp = r_pool.tile([1, TSL], F32R, tag="rc")
                        with nc.allow_low_precision(reason="softmax recip f32r"):
                            nc.vector.reciprocal(rcp[:], p_o[D:D + 1, :])
                        o_sb = o_pool.tile([P, TSL], F32R, tag="ob")
                        nc.vector.tensor_copy(o_sb[0:D, :], p_o[0:D, :])
                        p_b = ps_w.tile([P, TSL], F32, tag="pw")
                        nc.tensor.matmul(p_b[0:D, 0:SW], lhsT=ones_s,
                                         rhs=rcp[:, 0:SW], start=True, stop=True)
                        nc.tensor.matmul(p_b[0:D, SW:TSL], lhsT=ones_s,
                                         rhs=rcp[:, SW:TSL], start=True, stop=True)
                        nc.vector.tensor_mul(
                            OT[po:po + D, eth, t0:t0 + TSL],
                            o_sb[0:D, :], p_b[0:D, :])

            # ---- stage 4: output projection (no bias; host adds bp) ----
            with (
                tc.tile_pool(name="wop", bufs=1) as wo_pool,
                tc.tile_pool(name="ysb", bufs=4) as y_pool,
                tc.tile_pool(name="ps4", bufs=4, space="PSUM") as ps4,
            ):
                wo_s = wo_pool.tile([P, ET, C], F32R)
                nc.sync.dma_start(wo_s[:], wo_v)
                for ttt in range(T // P):
                    for jn in range(C // SW):
                        py = ps4.tile([P, SW], F32, tag="p4")
                        for ko in range(ET):
                            nc.tensor.matmul(
                                py[:], lhsT=OT[:, ko, ttt * P:(ttt + 1) * P],
                                rhs=wo_s[:, ko, jn * SW:(jn + 1) * SW],
                                start=(ko == 0), stop=(ko == ET - 1))
                        ys = y_pool.tile([P, SW], F32, tag="ys")
                        nc.scalar.copy(ys[:], py[:])
                        nc.sync.dma_start(
                            y_v[ttt * P:(ttt + 1) * P, jn * SW:(jn + 1) * SW],
                            ys[:])
    nc.compile()
    return nc


def _get_nc():
    if "nc" not in _CACHE:
        _CACHE["nc"] = _build()
    return _CACHE["nc"]


def kernel(x, Wq, Wk, Wv, Wp, bp):
    x = np.asarray(x, dtype=np.float32)
    Wq = np.asarray(Wq, dtype=np.float32)
    Wk = np.asarray(Wk, dtype=np.float32)
    Wv = np.asarray(Wv, dtype=np.float32)
    Wp = np.asarray(Wp, dtype=np.float32)
    bp = np.asarray(bp, dtype=np.float32)

    nc = _get_nc()
    in_maps = []
    for c in range(8):
        b, g = c // 2, c % 2
        hs = slice(g * NH, (g + 1) * NH)
        in_maps.append({
            "xt": np.ascontiguousarray(x[b].T),
            "wq": np.ascontiguousarray(Wq[hs].transpose(1, 0, 2).reshape(C, E)),
            "wk": np.ascontiguousarray(Wk[hs].transpose(1, 0, 2).reshape(C, E)),
            "wv": np.ascontiguousarray(Wv[hs].transpose(1, 0, 2).reshape(C, E)),
            "wo": np.ascontiguousarray(Wp[:, g * E:(g + 1) * E].T),
        })
    res = run_bass_kernel_spmd(nc, in_maps, core_ids=list(range(8)))
    _CACHE["last_result"] = res
    y = np.empty((B, T, C), dtype=np.float32)
    for b in range(B):
        y[b] = res.results[2 * b]["y"] + res.results[2 * b + 1]["y"] + bp
    return y


# revision 8
# speedup vs baseline: 1.3919x; 1.3919x over previous
"""Multi-head causal attention (B=4, T=2048, C=1024, H=16, D=64) on 8 TRN2 cores.

Sharding: core c = (batch b = c//2, head-group g = c%2 of 8 heads).
Per core (its batch, its 8 heads), all matmuls in bf16 with fp32 PSUM accum:
  QT/KT = W^T x^T              [E=512, T] head-major rows (bf16)
  V     = x Wv                 [T, E] natural orientation, augmented with a
                               64-wide block of ones columns per head
  per head, per 1024-wide query slab:
    scores^T = K_h Q_h^T       [s-chunk 128, t 1024] psum, causal-skipped
    P~ = exp(scores/sqrt(D))   bf16; diagonal chunks zeroed via affine_select
    O'^T = [V_h | 1s]^T P~     psum rows 0-63 = numerator, 64-127 = denominator
                               broadcast to 64 rows for free
    O^T = O'^T * recip(denom)  two DVE ops
  y_part = O^T^T Wo_g          [T, C] partial
Host: y_b = y_part(g=0) + y_part(g=1) + bp.

Attention emission is software-pipelined: the AV matmul for chunk j is emitted
after scores for chunk j+2, so the PE never waits head-of-line on exp/select.
"""

from collections import deque

import ml_dtypes
import numpy as np

import concourse.bacc as bacc
import concourse.mybir as mybir
import concourse.tile as tile
from concourse.bass_utils import run_bass_kernel_spmd

B, T, C, H, D = 4, 2048, 1024, 16, 64
NH = 8                 # heads per core
E = NH * D             # 512 per-core head width
P = 128
KO = C // P            # 8 contraction chunks for QKV proj
ET = E // P            # 4 e-tiles
SW = 512               # psum-bank width / stage-2 t-slab width
NSLAB = T // SW        # 4
TSL = 1024             # attention t-slab width (2 psum banks)
NSC = T // P           # 16 s-chunks
LOOKAHEAD = 2          # AV emission lag (chunks)
F32 = mybir.dt.float32
BF16 = mybir.dt.bfloat16
EXP = mybir.ActivationFunctionType.Exp
GE = mybir.AluOpType.is_ge
SCALE = float(D) ** -0.5
BF16NP = ml_dtypes.bfloat16

_CACHE: dict = {}


def _build():
    nc = bacc.Bacc("TRN2", target_bir_lowering=False, debug=False)
    xt_d = nc.dram_tensor("xt", [C, T], BF16, kind="ExternalInput")
    wq_d = nc.dram_tensor("wq", [C, E], BF16, kind="ExternalInput")
    wk_d = nc.dram_tensor("wk", [C, E], BF16, kind="ExternalInput")
    wv_d = nc.dram_tensor("wv", [C, E], BF16, kind="ExternalInput")
    wo_d = nc.dram_tensor("wo", [E, C], BF16, kind="ExternalInput")
    y_d = nc.dram_tensor("y", [T, C], F32, kind="ExternalOutput")

    xt_v = xt_d.ap().rearrange("(ko p) t -> p ko t", p=P)
    wq_v = wq_d.ap().rearrange("(ko p) e -> p ko e", p=P)
    wk_v = wk_d.ap().rearrange("(ko p) e -> p ko e", p=P)
    wv_v = wv_d.ap().rearrange("(ko p) e -> p ko e", p=P)
    wo_v = wo_d.ap().rearrange("(ko p) j -> p ko j", p=P)
    y_v = y_d.ap()

    with tile.TileContext(nc) as tc:
        with (
            tc.tile_pool(name="qkv", bufs=1) as qkv_pool,
            tc.tile_pool(name="vsb", bufs=1) as v_pool,
            tc.tile_pool(name="otp", bufs=1) as ot_pool,
        ):
            QT = qkv_pool.tile([P, ET, T], BF16)
            KT = qkv_pool.tile([P, ET, T], BF16)
            # V augmented: cols 0-63 = head data, 64-127 = ones
            Vsb = v_pool.tile([P, NSC, NH, P], BF16)
            OT = ot_pool.tile([P, ET, T], BF16)
            ones_t = v_pool.tile([P, 1], F32)
            nc.gpsimd.memset(ones_t[:], 1.0)
            nc.vector.tensor_copy(
                Vsb[:, :, :, D:P],
                ones_t[:, 0:1, None, None].to_broadcast((P, NSC, NH, D)))

            # ---- stage 1+2: load inputs, QKV projections ----
            with (
                tc.tile_pool(name="wgt", bufs=1) as w_pool,
                tc.tile_pool(name="xsl", bufs=2) as x_pool,
                tc.tile_pool(name="ps2", bufs=8, space="PSUM") as ps2,
            ):
                wq_s = w_pool.tile([P, KO, E], BF16)
                wk_s = w_pool.tile([P, KO, E], BF16)
                wv_s = w_pool.tile([P, KO, E], BF16)
                nc.sync.dma_start(wq_s[:], wq_v)
                nc.sync.dma_start(wk_s[:], wk_v)
                nc.sync.dma_start(wv_s[:], wv_v)

                for sl in range(NSLAB):
                    xs = x_pool.tile([P, KO, SW], BF16, tag="xs")
                    nc.sync.dma_start(xs[:], xt_v[:, :, sl * SW:(sl + 1) * SW])
                    for et in range(ET):
                        pq = ps2.tile([P, SW], F32, tag="p2")
                        for ko in range(KO):
                            nc.tensor.matmul(
                                pq[:], lhsT=wq_s[:, ko, et * P:(et + 1) * P],
                                rhs=xs[:, ko, :],
                                start=(ko == 0), stop=(ko == KO - 1))
                        nc.vector.tensor_copy(QT[:, et, sl * SW:(sl + 1) * SW], pq[:])
                        pk = ps2.tile([P, SW], F32, tag="p2")
                        for ko in range(KO):
                            nc.tensor.matmul(
                                pk[:], lhsT=wk_s[:, ko, et * P:(et + 1) * P],
                                rhs=xs[:, ko, :],
                                start=(ko == 0), stop=(ko == KO - 1))
                        nc.vector.tensor_copy(KT[:, et, sl * SW:(sl + 1) * SW], pk[:])
                    for si in range(SW // P):
                        pv = ps2.tile([P, E], F32, tag="p2")
                        for ko in range(KO):
                            nc.tensor.matmul(
                                pv[:], lhsT=xs[:, ko, si * P:(si + 1) * P],
                                rhs=wv_s[:, ko, :],
                                start=(ko == 0), stop=(ko == KO - 1))
                        st = sl * (SW // P) + si
                        nc.vector.tensor_copy(
                            Vsb[:, st, :, 0:D],
                            pv[:].rearrange("p (h d) -> p h d", d=D))

            # ---- stage 3: attention, software-pipelined emission ----
            with (
                tc.tile_pool(name="ptl", bufs=5) as pt_pool,
                tc.tile_pool(name="rcs", bufs=2) as r_pool,
                tc.tile_pool(name="psw", bufs=2, space="PSUM") as ps_w,
                tc.tile_pool(name="pso", bufs=2, space="PSUM") as ps_o,
            ):
                av_q = deque()  # (emit_fn, norm_fn_or_None)

                def flush(n_keep):
                    while len(av_q) > n_keep:
                        emit, norm = av_q.popleft()
                        emit()
                        if norm is not None:
                            norm()

                for h in range(NH):
                    eth = h // 2
                    po = (h % 2) * D
                    for tt2 in range(T // TSL):
                        t0 = tt2 * TSL
                        n_sc = (t0 + TSL) // P
                        last_lo = (t0 + SW) // P - 1
                        p_o = ps_o.tile([P, TSL], F32, tag="po")
                        for sc in range(n_sc):
                            dlt = sc * P - t0
                            lo_valid = dlt < SW
                            lo = 0 if lo_valid else SW
                            p_w = ps_w.tile([P, TSL], F32, tag="pw")
                            kk = KT[po:po + D, eth, sc * P:(sc + 1) * P]
                            if lo_valid:
                                nc.tensor.matmul(
                                    p_w[:, 0:SW], lhsT=kk,
                                    rhs=QT[po:po + D, eth, t0:t0 + SW],
                                    start=True, stop=True)
                            nc.tensor.matmul(
                                p_w[:, SW:TSL], lhsT=kk,
                                rhs=QT[po:po + D, eth, t0 + SW:t0 + TSL],
                                start=True, stop=True)
                            p_t = pt_pool.tile([P, TSL], BF16, tag="pt")
                            nc.scalar.activation(
                                p_t[:, lo:TSL], p_w[:, lo:TSL], EXP, scale=SCALE)
                            if dlt >= 0:
                                w_hi = min(dlt + P, TSL)
                                if w_hi > lo:
                                    nc.gpsimd.affine_select(
                                        out=p_t[:, lo:w_hi], in_=p_t[:, lo:w_hi],
                                        pattern=[[1, w_hi - lo]], compare_op=GE,
                                        fill=0.0, base=lo - dlt,
                                        channel_multiplier=-1)

                            def mk_av(p_o=p_o, p_t=p_t, sc=sc, h=h,
                                      lo_valid=lo_valid, last_lo=last_lo,
                                      n_sc=n_sc):
                                def emit():
                                    vv = Vsb[:, sc, h, :]
                                    if lo_valid:
                                        nc.tensor.matmul(
                                            p_o[:, 0:SW], lhsT=vv,
                                            rhs=p_t[:, 0:SW],
                                            start=(sc == 0),
                                            stop=(sc == last_lo))
                                    nc.tensor.matmul(
                                        p_o[:, SW:TSL], lhsT=vv,
                                        rhs=p_t[:, SW:TSL],
                                        start=(sc == 0), stop=(sc == n_sc - 1))
                                return emit

                            def mk_norm(p_o=p_o, eth=eth, po=po, t0=t0):
                                def emit():
                                    rcp = r_pool.tile([P, TSL], F32, tag="rc")
                                    nc.vector.reciprocal(
                                        rcp[0:D, :], p_o[D:P, :])
                                    nc.vector.tensor_mul(
                                        OT[po:po + D, eth, t0:t0 + TSL],
                                        p_o[0:D, :], rcp[0:D, :])
                                return emit

                            is_last = sc == n_sc - 1
                            av_q.append((mk_av(), mk_norm() if is_last else None))
                            flush(LOOKAHEAD)
                flush(0)

            # ---- stage 4: output projection (no bias; host adds bp) ----
            with (
                tc.tile_pool(name="wop", bufs=1) as wo_pool,
                tc.tile_pool(name="ysb", bufs=4) as y_pool,
                tc.tile_pool(name="ps4", bufs=4, space="PSUM") as ps4,
            ):
                wo_s = wo_pool.tile([P, ET, C], BF16)
                nc.sync.dma_start(wo_s[:], wo_v)
                for ttt in range(T // P):
                    for jn in range(C // SW):
                        py = ps4.tile([P, SW], F32, tag="p4")
                        for ko in range(ET):
                            nc.tensor.matmul(
                                py[:], lhsT=OT[:, ko, ttt * P:(ttt + 1) * P],
                                rhs=wo_s[:, ko, jn * SW:(jn + 1) * SW],
                                start=(ko == 0), stop=(ko == ET - 1))
                        ys = y_pool.tile([P, SW], F32, tag="ys")
                        nc.vector.tensor_copy(ys[:], py[:])
                        nc.sync.dma_start(
                            y_v[ttt * P:(ttt + 1) * P, jn * SW:(jn + 1) * SW],
                            ys[:])
    nc.compile()
    return nc


def _get_nc():
    if "nc" not in _CACHE:
        _CACHE["nc"] = _build()
    return _CACHE["nc"]


def kernel(x, Wq, Wk, Wv, Wp, bp):
    x = np.asarray(x, dtype=np.float32)
    Wq = np.asarray(Wq, dtype=np.float32)
    Wk = np.asarray(Wk, dtype=np.float32)
    Wv = np.asarray(Wv, dtype=np.float32)
    Wp = np.asarray(Wp, dtype=np.float32)
    bp = np.asarray(bp, dtype=np.float32)

    nc = _get_nc()
    in_maps = []
    for c in range(8):
        b, g = c // 2, c % 2
        hs = slice(g * NH, (g + 1) * NH)
        in_maps.append({
            "xt": np.ascontiguousarray(x[b].T).astype(BF16NP),
            "wq": Wq[hs].transpose(1, 0, 2).reshape(C, E).astype(BF16NP),
            "wk": Wk[hs].transpose(1, 0, 2).reshape(C, E).astype(BF16NP),
            "wv": Wv[hs].transpose(1, 0, 2).reshape(C, E).astype(BF16NP),
            "wo": np.ascontiguousarray(Wp[:, g * E:(g + 1) * E].T).astype(BF16NP),
        })
    res = run_bass_kernel_spmd(nc, in_maps, core_ids=list(range(8)))
    _CACHE["last_result"] = res
    y = np.empty((B, T, C), dtype=np.float32)
    for b in range(B):
        y[b] = res.results[2 * b]["y"] + res.results[2 * b + 1]["y"] + bp
    return y


# revision 11
# speedup vs baseline: 2.0065x; 1.4416x over previous
"""Multi-head causal attention (B=4, T=2048, C=1024, H=16, D=64) on 8 TRN2 cores.

Sharding: core c = (batch b = c//2, head-group g = c%2 of 8 heads).
Per core (its batch, its 8 heads), all matmuls in bf16 with fp32 PSUM accum:
  QT/KT = W^T x^T              [E=512, T] head-major rows (bf16)
  V     = x Wv                 [T, E] natural orientation, augmented with a
                               64-wide block of ones columns per head
  per head, per 1024-wide query slab:
    scores^T = K_h Q_h^T       [s-chunk 128, t 1024] psum, causal-skipped
    P~ = exp(scores/sqrt(D))   bf16; diagonal chunks zeroed via affine_select
    O'^T = [V_h | 1s]^T P~     psum rows 0-63 = numerator, 64-127 = denominator
                               broadcast to 64 rows for free
    O^T = O'^T * recip(denom)  two DVE ops per 512-half
  y_part = O^T^T Wo_g          [T, C] partial
Host: y_b = y_part(g=0) + y_part(g=1) + bp.

Scheduling: attention is ACT(exp)-paced, so stage-2 (QKV projection) slabs 2-3
and the first half of the output projection are emitted as small "filler"
pieces interleaved between attention chunks to keep the PE busy while ACT
works. The AV matmul for chunk j is emitted after scores for chunk j+2 so the
PE never waits head-of-line on exp/select.
"""

from collections import deque

import ml_dtypes
import numpy as np

import concourse.bacc as bacc
import concourse.mybir as mybir
import concourse.tile as tile
from concourse.bass_utils import run_bass_kernel_spmd

B, T, C, H, D = 4, 2048, 1024, 16, 64
NH = 8                 # heads per core
E = NH * D             # 512 per-core head width
P = 128
KO = C // P            # 8 contraction chunks for QKV proj
ET = E // P            # 4 e-tiles
SW = 512               # psum-bank width / stage-2 t-slab width
NSLAB = T // SW        # 4
TSL = 1024             # attention t-slab width (2 psum banks)
NSC = T // P           # 16 s-chunks
LOOKAHEAD = 2          # AV emission lag (chunks)
F32 = mybir.dt.float32
BF16 = mybir.dt.bfloat16
EXP = mybir.ActivationFunctionType.Exp
GE = mybir.AluOpType.is_ge
SCALE = float(D) ** -0.5
BF16NP = ml_dtypes.bfloat16

_CACHE: dict = {}


def _build():
    nc = bacc.Bacc("TRN2", target_bir_lowering=False, debug=False)
    xt_d = nc.dram_tensor("xt", [C, T], BF16, kind="ExternalInput")
    wq_d = nc.dram_tensor("wq", [C, E], BF16, kind="ExternalInput")
    wk_d = nc.dram_tensor("wk", [C, E], BF16, kind="ExternalInput")
    wv_d = nc.dram_tensor("wv", [C, E], BF16, kind="ExternalInput")
    wo_d = nc.dram_tensor("wo", [E, C], BF16, kind="ExternalInput")
    y_d = nc.dram_tensor("y", [T, C], F32, kind="ExternalOutput")

    xt_v = xt_d.ap().rearrange("(ko p) t -> p ko t", p=P)
    wq_v = wq_d.ap().rearrange("(ko p) e -> p ko e", p=P)
    wk_v = wk_d.ap().rearrange("(ko p) e -> p ko e", p=P)
    wv_v = wv_d.ap().rearrange("(ko p) e -> p ko e", p=P)
    wo_v = wo_d.ap().rearrange("(ko p) j -> p ko j", p=P)
    y_v = y_d.ap()

    with tile.TileContext(nc) as tc:
        with (
            tc.tile_pool(name="qkv", bufs=1) as qkv_pool,
            tc.tile_pool(name="vsb", bufs=1) as v_pool,
            tc.tile_pool(name="otp", bufs=1) as ot_pool,
            tc.tile_pool(name="wgt", bufs=1) as w_pool,
            tc.tile_pool(name="xsl", bufs=2) as x_pool,
            tc.tile_pool(name="wop", bufs=1) as wo_pool,
            tc.tile_pool(name="ptl", bufs=5) as pt_pool,
            tc.tile_pool(name="rcs", bufs=4) as r_pool,
            tc.tile_pool(name="ysb", bufs=4) as y_pool,
            tc.tile_pool(name="psu", bufs=4, space="PSUM") as ps_u,
            tc.tile_pool(name="psw", bufs=2, space="PSUM") as ps_w,
        ):
            QT = qkv_pool.tile([P, ET, T], BF16)
            KT = qkv_pool.tile([P, ET, T], BF16)
            # V augmented: cols 0-63 = head data, 64-127 = ones
            Vsb = v_pool.tile([P, NSC, NH, P], BF16)
            OT = ot_pool.tile([P, ET, T], BF16)
            ones_t = v_pool.tile([P, 1], F32)
            nc.gpsimd.memset(ones_t[:], 1.0)
            nc.vector.tensor_copy(
                Vsb[:, :, :, D:P],
                ones_t[:, 0:1, None, None].to_broadcast((P, NSC, NH, D)))

            wq_s = w_pool.tile([P, KO, E], BF16)
            wk_s = w_pool.tile([P, KO, E], BF16)
            wv_s = w_pool.tile([P, KO, E], BF16)
            wo_s = wo_pool.tile([P, ET, C], BF16)
            nc.sync.dma_start(wq_s[:], wq_v)
            nc.sync.dma_start(wk_s[:], wk_v)
            nc.sync.dma_start(wv_s[:], wv_v)

            # ---- stage-2 pieces: one slab = 1 DMA + 12 psum-fill pieces ----
            def stage2_pieces(sl):
                xs = x_pool.tile([P, KO, SW], BF16, tag="xs")

                def dma():
                    nc.sync.dma_start(xs[:], xt_v[:, :, sl * SW:(sl + 1) * SW])
                yield dma
                for et in range(ET):
                    def qk_fill(et=et, xs=xs, w=wq_s, dst=QT):
                        pq = ps_u.tile([P, SW], F32, tag="ps")
                        for ko in range(KO):
                            nc.tensor.matmul(
                                pq[:], lhsT=w[:, ko, et * P:(et + 1) * P],
                                rhs=xs[:, ko, :],
                                start=(ko == 0), stop=(ko == KO - 1))
                        nc.vector.tensor_copy(
                            dst[:, et, sl * SW:(sl + 1) * SW], pq[:])
                    yield qk_fill
                    def k_fill(et=et, xs=xs, w=wk_s, dst=KT):
                        pk = ps_u.tile([P, SW], F32, tag="ps")
                        for ko in range(KO):
                            nc.tensor.matmul(
                                pk[:], lhsT=w[:, ko, et * P:(et + 1) * P],
                                rhs=xs[:, ko, :],
                                start=(ko == 0), stop=(ko == KO - 1))
                        nc.vector.tensor_copy(
                            dst[:, et, sl * SW:(sl + 1) * SW], pk[:])
                    yield k_fill
                for si in range(SW // P):
                    def v_fill(si=si, xs=xs):
                        pv = ps_u.tile([P, E], F32, tag="ps")
                        for ko in range(KO):
                            nc.tensor.matmul(
                                pv[:], lhsT=xs[:, ko, si * P:(si + 1) * P],
                                rhs=wv_s[:, ko, :],
                                start=(ko == 0), stop=(ko == KO - 1))
                        st = sl * (SW // P) + si
                        nc.vector.tensor_copy(
                            Vsb[:, st, :, 0:D],
                            pv[:].rearrange("p (h d) -> p h d", d=D))
                    yield v_fill

            # ---- projection pieces: one = 4 matmuls + copy + DMA out ----
            def proj_pieces(ttt_range):
                for ttt in ttt_range:
                    for jn in range(C // SW):
                        def piece(ttt=ttt, jn=jn):
                            py = ps_u.tile([P, SW], F32, tag="ps")
                            for ko in range(ET):
                                nc.tensor.matmul(
                                    py[:],
                                    lhsT=OT[:, ko, ttt * P:(ttt + 1) * P],
                                    rhs=wo_s[:, ko, jn * SW:(jn + 1) * SW],
                                    start=(ko == 0), stop=(ko == ET - 1))
                            ys = y_pool.tile([P, SW], F32, tag="ys")
                            nc.vector.tensor_copy(ys[:], py[:])
                            nc.sync.dma_start(
                                y_v[ttt * P:(ttt + 1) * P,
                                    jn * SW:(jn + 1) * SW], ys[:])
                        yield piece

            # ---- emit: slabs 0-1 up front ----
            for sl in (0, 1):
                for piece in stage2_pieces(sl):
                    piece()

            # ---- attention with filler interleaving ----
            av_q = deque()

            def flush(n_keep):
                while len(av_q) > n_keep:
                    emit, norm = av_q.popleft()
                    emit()
                    if norm is not None:
                        norm()

            def attention(blocks, fillers, fill_every):
                fillers = deque(fillers)
                job = 0
                for h, tt2 in blocks:
                    eth = h // 2
                    po = (h % 2) * D
                    t0 = tt2 * TSL
                    n_sc = (t0 + TSL) // P
                    last_lo = (t0 + SW) // P - 1
                    p_oA = ps_u.tile([P, SW], F32, tag="ps")
                    p_oB = ps_u.tile([P, SW], F32, tag="ps")
                    for sc in range(n_sc):
                        dlt = sc * P - t0
                        lo_valid = dlt < SW
                        lo = 0 if lo_valid else SW
                        p_w = ps_w.tile([P, TSL], F32, tag="pw")
                        kk = KT[po:po + D, eth, sc * P:(sc + 1) * P]
                        if lo_valid:
                            nc.tensor.matmul(
                                p_w[:, 0:SW], lhsT=kk,
                                rhs=QT[po:po + D, eth, t0:t0 + SW],
                                start=True, stop=True)
                        nc.tensor.matmul(
                            p_w[:, SW:TSL], lhsT=kk,
                            rhs=QT[po:po + D, eth, t0 + SW:t0 + TSL],
                            start=True, stop=True)
                        p_t = pt_pool.tile([P, TSL], BF16, tag="pt")
                        e0 = max(lo, dlt)  # cols < dlt are zeroed by select
                        nc.scalar.activation(
                            p_t[:, e0:TSL], p_w[:, e0:TSL], EXP, scale=SCALE)
                        if dlt >= 0:
                            w_hi = min(dlt + P, TSL)
                            if w_hi > lo:
                                nc.gpsimd.affine_select(
                                    out=p_t[:, lo:w_hi], in_=p_t[:, lo:w_hi],
                                    pattern=[[1, w_hi - lo]], compare_op=GE,
                                    fill=0.0, base=lo - dlt,
                                    channel_multiplier=-1)

                        def mk_av(p_oA=p_oA, p_oB=p_oB, p_t=p_t, sc=sc, h=h,
                                  lo_valid=lo_valid, last_lo=last_lo,
                                  n_sc=n_sc):
                            def emit():
                                vv = Vsb[:, sc, h, :]
                                if lo_valid:
                                    nc.tensor.matmul(
                                        p_oA[:], lhsT=vv, rhs=p_t[:, 0:SW],
                                        start=(sc == 0), stop=(sc == last_lo))
                                nc.tensor.matmul(
                                    p_oB[:], lhsT=vv, rhs=p_t[:, SW:TSL],
                                    start=(sc == 0), stop=(sc == n_sc - 1))
                            return emit

                        def mk_norm(p_oA=p_oA, p_oB=p_oB, eth=eth, po=po,
                                    t0=t0):
                            def emit():
                                for half, p_o in ((0, p_oA), (1, p_oB)):
                                    ta = t0 + half * SW
                                    rcp = r_pool.tile([P, SW], F32, tag="rc")
                                    dsb = r_pool.tile([P, SW], F32, tag="db")
                                    nc.vector.tensor_copy(
                                        dsb[0:D, :], p_o[D:P, :])
                                    nc.vector.reciprocal_approx_fast(
                                        out=rcp[0:D, :], in_=dsb[0:D, :])
                                    nc.vector.tensor_mul(
                                        OT[po:po + D, eth, ta:ta + SW],
                                        p_o[0:D, :], rcp[0:D, :])
                            return emit

                        is_last = sc == n_sc - 1
                        av_q.append((mk_av(), mk_norm() if is_last else None))
                        flush(LOOKAHEAD)
                        job += 1
                        if fillers and job % fill_every == 0:
                            fillers.popleft()()
                while fillers:
                    fillers.popleft()()

            fill_b = list(stage2_pieces(2)) + list(stage2_pieces(3))
            attention([(h, 0) for h in range(NH)], fill_b, 2)

            def wo_dma():
                nc.sync.dma_start(wo_s[:], wo_v)
            fill_c = [wo_dma] + list(proj_pieces(range(0, T // P // 2)))
            attention([(h, 1) for h in range(NH)], fill_c, 7)
            flush(0)

            # ---- tail: remaining projection ----
            for piece in proj_pieces(range(T // P // 2, T // P)):
                piece()
    nc.compile()
    return nc


def _get_nc():
    if "nc" not in _CACHE:
        _CACHE["nc"] = _build()
    return _CACHE["nc"]


def kernel(x, Wq, Wk, Wv, Wp, bp):
    x = np.asarray(x, dtype=np.float32)
    Wq = np.asarray(Wq, dtype=np.float32)
    Wk = np.asarray(Wk, dtype=np.float32)
    Wv = np.asarray(Wv, dtype=np.float32)
    Wp = np.asarray(Wp, dtype=np.float32)
    bp = np.asarray(bp, dtype=np.float32)

    nc = _get_nc()
    in_maps = []
    for c in range(8):
        b, g = c // 2, c % 2
        hs = slice(g * NH, (g + 1) * NH)
        in_maps.append({
            "xt": np.ascontiguousarray(x[b].T).astype(BF16NP),
            "wq": Wq[hs].transpose(1, 0, 2).reshape(C, E).astype(BF16NP),
            "wk": Wk[hs].transpose(1, 0, 2).reshape(C, E).astype(BF16NP),
            "wv": Wv[hs].transpose(1, 0, 2).reshape(C, E).astype(BF16NP),
            "wo": np.ascontiguousarray(Wp[:, g * E:(g + 1) * E].T).astype(BF16NP),
        })
    res = run_bass_kernel_spmd(nc, in_maps, core_ids=list(range(8)))
    _CACHE["last_result"] = res
    y = np.empty((B, T, C), dtype=np.float32)
    for b in range(B):
        y[b] = res.results[2 * b]["y"] + res.results[2 * b + 1]["y"] + bp
    return y


# revision 12
# speedup vs baseline: 2.0225x; 1.0080x over previous
"""Multi-head causal attention (B=4, T=2048, C=1024, H=16, D=64) on 8 TRN2 cores.

Sharding: core c = (batch b = c//2, head-group g = c%2 of 8 heads).
Per core (its batch, its 8 heads), all matmuls in bf16 with fp32 PSUM accum:
  QT/KT = W^T x^T              [E=512, T] head-major rows (bf16)
  V     = x Wv                 [T, E] natural orientation, augmented with a
                               64-wide block of ones columns per head
  per head, per 1024-wide query slab:
    scores^T = K_h Q_h^T       [s-chunk 128, t 1024] psum, causal-skipped
    P~ = exp(scores/sqrt(D))   bf16; diagonal chunks zeroed via affine_select
    O'^T = [V_h | 1s]^T P~     psum rows 0-63 = numerator, 64-127 = denominator
                               broadcast to 64 rows for free
    O^T = O'^T * recip(denom)  two DVE ops per 512-half
  y_part = O^T^T Wo_g          [T, C] partial
Host: y_b = y_part(g=0) + y_part(g=1) + bp.

Scheduling: attention is ACT(exp)-paced, so stage-2 (QKV projection) slabs 2-3
and the first half of the output projection are emitted as small "filler"
pieces interleaved between attention chunks to keep the PE busy while ACT
works. The AV matmul for chunk j is emitted after scores for chunk j+2 so the
PE never waits head-of-line on exp/select.
"""

from collections import deque

import ml_dtypes
import numpy as np

import concourse.bacc as bacc
import concourse.mybir as mybir
import concourse.tile as tile
from concourse.bass_utils import run_bass_kernel_spmd

B, T, C, H, D = 4, 2048, 1024, 16, 64
NH = 8                 # heads per core
E = NH * D             # 512 per-core head width
P = 128
KO = C // P            # 8 contraction chunks for QKV proj
ET = E // P            # 4 e-tiles
SW = 512               # psum-bank width / stage-2 t-slab width
NSLAB = T // SW        # 4
TSL = 1024             # attention t-slab width (2 psum banks)
NSC = T // P           # 16 s-chunks
LOOKAHEAD = 2          # AV emission lag (chunks)
F32 = mybir.dt.float32
BF16 = mybir.dt.bfloat16
EXP = mybir.ActivationFunctionType.Exp
GE = mybir.AluOpType.is_ge
SCALE = float(D) ** -0.5
BF16NP = ml_dtypes.bfloat16

_CACHE: dict = {}


def _build():
    nc = bacc.Bacc("TRN2", target_bir_lowering=False, debug=False)
    xt_d = nc.dram_tensor("xt", [C, T], BF16, kind="ExternalInput")
    wq_d = nc.dram_tensor("wq", [C, E], BF16, kind="ExternalInput")
    wk_d = nc.dram_tensor("wk", [C, E], BF16, kind="ExternalInput")
    wv_d = nc.dram_tensor("wv", [C, E], BF16, kind="ExternalInput")
    wo_d = nc.dram_tensor("wo", [E, C], BF16, kind="ExternalInput")
    y_d = nc.dram_tensor("y", [T, C], F32, kind="ExternalOutput")

    xt_v = xt_d.ap().rearrange("(ko p) t -> p ko t", p=P)
    wq_v = wq_d.ap().rearrange("(ko p) e -> p ko e", p=P)
    wk_v = wk_d.ap().rearrange("(ko p) e -> p ko e", p=P)
    wv_v = wv_d.ap().rearrange("(ko p) e -> p ko e", p=P)
    wo_v = wo_d.ap().rearrange("(ko p) j -> p ko j", p=P)
    y_v = y_d.ap()

    with tile.TileContext(nc) as tc:
        with (
            tc.tile_pool(name="qkv", bufs=1) as qkv_pool,
            tc.tile_pool(name="vsb", bufs=1) as v_pool,
            tc.tile_pool(name="otp", bufs=1) as ot_pool,
            tc.tile_pool(name="wgt", bufs=1) as w_pool,
            tc.tile_pool(name="xsl", bufs=2) as x_pool,
            tc.tile_pool(name="wop", bufs=1) as wo_pool,
            tc.tile_pool(name="ptl", bufs=5) as pt_pool,
            tc.tile_pool(name="rcs", bufs=4) as r_pool,
            tc.tile_pool(name="ysb", bufs=4) as y_pool,
            tc.tile_pool(name="psu", bufs=4, space="PSUM") as ps_u,
            tc.tile_pool(name="psw", bufs=2, space="PSUM") as ps_w,
        ):
            QT = qkv_pool.tile([P, ET, T], BF16)
            KT = qkv_pool.tile([P, ET, T], BF16)
            # V augmented: cols 0-63 = head data, 64-127 = ones
            Vsb = v_pool.tile([P, NSC, NH, P], BF16)
            OT = ot_pool.tile([P, ET, T], BF16)
            ones_t = v_pool.tile([P, 1], F32)
            nc.gpsimd.memset(ones_t[:], 1.0)
            nc.vector.tensor_copy(
                Vsb[:, :, :, D:P],
                ones_t[:, 0:1, None, None].to_broadcast((P, NSC, NH, D)))

            wq_s = w_pool.tile([P, KO, E], BF16)
            wk_s = w_pool.tile([P, KO, E], BF16)
            wv_s = w_pool.tile([P, KO, E], BF16)
            wo_s = wo_pool.tile([P, ET, C], BF16)

            # ---- stage-2 pieces: one slab = 1 DMA + 12 psum-fill pieces ----
            def stage2_pieces(sl):
                xs = x_pool.tile([P, KO, SW], BF16, tag="xs")

                def dma():
                    nc.sync.dma_start(xs[:], xt_v[:, :, sl * SW:(sl + 1) * SW])
                yield dma
                for et in range(ET):
                    def qk_fill(et=et, xs=xs, w=wq_s, dst=QT):
                        pq = ps_u.tile([P, SW], F32, tag="ps")
                        for ko in range(KO):
                            nc.tensor.matmul(
                                pq[:], lhsT=w[:, ko, et * P:(et + 1) * P],
                                rhs=xs[:, ko, :],
                                start=(ko == 0), stop=(ko == KO - 1))
                        nc.vector.tensor_copy(
                            dst[:, et, sl * SW:(sl + 1) * SW], pq[:])
                    yield qk_fill
                    def k_fill(et=et, xs=xs, w=wk_s, dst=KT):
                        pk = ps_u.tile([P, SW], F32, tag="ps")
                        for ko in range(KO):
                            nc.tensor.matmul(
                                pk[:], lhsT=w[:, ko, et * P:(et + 1) * P],
                                rhs=xs[:, ko, :],
                                start=(ko == 0), stop=(ko == KO - 1))
                        nc.vector.tensor_copy(
                            dst[:, et, sl * SW:(sl + 1) * SW], pk[:])
                    yield k_fill
                for si in range(SW // P):
                    def v_fill(si=si, xs=xs):
                        pv = ps_u.tile([P, E], F32, tag="ps")
                        for ko in range(KO):
                            nc.tensor.matmul(
                                pv[:], lhsT=xs[:, ko, si * P:(si + 1) * P],
                                rhs=wv_s[:, ko, :],
                                start=(ko == 0), stop=(ko == KO - 1))
                        st = sl * (SW // P) + si
                        nc.vector.tensor_copy(
                            Vsb[:, st, :, 0:D],
                            pv[:].rearrange("p (h d) -> p h d", d=D))
                    yield v_fill

            # ---- projection pieces: one = 4 matmuls + copy + DMA out ----
            def proj_pieces(ttt_range):
                for ttt in ttt_range:
                    for jn in range(C // SW):
                        def piece(ttt=ttt, jn=jn):
                            py = ps_u.tile([P, SW], F32, tag="ps")
                            for ko in range(ET):
                                nc.tensor.matmul(
                                    py[:],
                                    lhsT=OT[:, ko, ttt * P:(ttt + 1) * P],
                                    rhs=wo_s[:, ko, jn * SW:(jn + 1) * SW],
                                    start=(ko == 0), stop=(ko == ET - 1))
                            ys = y_pool.tile([P, SW], F32, tag="ys")
                            nc.vector.tensor_copy(ys[:], py[:])
                            nc.sync.dma_start(
                                y_v[ttt * P:(ttt + 1) * P,
                                    jn * SW:(jn + 1) * SW], ys[:])
                        yield piece

            # ---- emit: slabs 0-1 up front (x slab DMA first, then weights
            # in first-use order so the first Q fill starts ASAP) ----
            gens = [stage2_pieces(0), stage2_pieces(1)]
            next(gens[0])()  # slab0 x DMA
            nc.sync.dma_start(wq_s[:], wq_v)
            next(gens[1])()  # slab1 x DMA
            nc.sync.dma_start(wk_s[:], wk_v)
            nc.sync.dma_start(wv_s[:], wv_v)
            for g in gens:
                for piece in g:
                    piece()

            # ---- attention with filler interleaving ----
            av_q = deque()

            def flush(n_keep):
                while len(av_q) > n_keep:
                    emit, norm = av_q.popleft()
                    emit()
                    if norm is not None:
                        norm()

            def attention(blocks, fillers, fill_every):
                fillers = deque(fillers)
                job = 0
                for h, tt2 in blocks:
                    eth = h // 2
                    po = (h % 2) * D
                    t0 = tt2 * TSL
                    n_sc = (t0 + TSL) // P
                    last_lo = (t0 + SW) // P - 1
                    p_oA = ps_u.tile([P, SW], F32, tag="ps")
                    p_oB = ps_u.tile([P, SW], F32, tag="ps")
                    for sc in range(n_sc):
                        dlt = sc * P - t0
                        lo_valid = dlt < SW
                        lo = 0 if lo_valid else SW
                        p_w = ps_w.tile([P, TSL], F32, tag="pw")
                        kk = KT[po:po + D, eth, sc * P:(sc + 1) * P]
                        if lo_valid:
                            nc.tensor.matmul(
                                p_w[:, 0:SW], lhsT=kk,
                                rhs=QT[po:po + D, eth, t0:t0 + SW],
                                start=True, stop=True)
                        nc.tensor.matmul(
                            p_w[:, SW:TSL], lhsT=kk,
                            rhs=QT[po:po + D, eth, t0 + SW:t0 + TSL],
                            start=True, stop=True)
                        p_t = pt_pool.tile([P, TSL], BF16, tag="pt")
                        e0 = max(lo, dlt)  # cols < dlt are zeroed by select
                        nc.scalar.activation(
                            p_t[:, e0:TSL], p_w[:, e0:TSL], EXP, scale=SCALE)
                        if dlt >= 0:
                            w_hi = min(dlt + P, TSL)
                            if w_hi > lo:
                                nc.gpsimd.affine_select(
                                    out=p_t[:, lo:w_hi], in_=p_t[:, lo:w_hi],
                                    pattern=[[1, w_hi - lo]], compare_op=GE,
                                    fill=0.0, base=lo - dlt,
                                    channel_multiplier=-1)

                        def mk_av(p_oA=p_oA, p_oB=p_oB, p_t=p_t, sc=sc, h=h,
                                  lo_valid=lo_valid, last_lo=last_lo,
                                  n_sc=n_sc):
                            def emit():
                                vv = Vsb[:, sc, h, :]
                                if lo_valid:
                                    nc.tensor.matmul(
                                        p_oA[:], lhsT=vv, rhs=p_t[:, 0:SW],
                                        start=(sc == 0), stop=(sc == last_lo))
                                nc.tensor.matmul(
                                    p_oB[:], lhsT=vv, rhs=p_t[:, SW:TSL],
                                    start=(sc == 0), stop=(sc == n_sc - 1))
                            return emit

                        def mk_norm(p_oA=p_oA, p_oB=p_oB, eth=eth, po=po,
                                    t0=t0):
                            def emit():
                                for half, p_o in ((0, p_oA), (1, p_oB)):
                                    ta = t0 + half * SW
                                    rcp = r_pool.tile([P, SW], F32, tag="rc")
                                    dsb = r_pool.tile([P, SW], F32, tag="db")
                                    nc.vector.tensor_copy(
                                        dsb[0:D, :], p_o[D:P, :])
                                    nc.vector.reciprocal_approx_fast(
                                        out=rcp[0:D, :], in_=dsb[0:D, :])
                                    nc.vector.tensor_mul(
                                        OT[po:po + D, eth, ta:ta + SW],
                                        p_o[0:D, :], rcp[0:D, :])
                            return emit

                        is_last = sc == n_sc - 1
                        av_q.append((mk_av(), mk_norm() if is_last else None))
                        flush(LOOKAHEAD)
                        job += 1
                        if fillers and job % fill_every == 0:
                            fillers.popleft()()
                while fillers:
                    fillers.popleft()()

            fill_b = list(stage2_pieces(2)) + list(stage2_pieces(3))
            attention([(h, 0) for h in range(NH)], fill_b, 2)

            def wo_dma():
                nc.sync.dma_start(wo_s[:], wo_v)
            fill_c = [wo_dma] + list(proj_pieces(range(0, T // P // 2)))
            attention([(h, 1) for h in range(NH)], fill_c, 7)
            flush(0)

            # ---- tail: remaining projection ----
            for piece in proj_pieces(range(T // P // 2, T // P)):
                piece()
    nc.compile()
    return nc


def _get_nc():
    if "nc" not in _CACHE:
        _CACHE["nc"] = _build()
    return _CACHE["nc"]


def kernel(x, Wq, Wk, Wv, Wp, bp):
    x = np.asarray(x, dtype=np.float32)
    Wq = np.asarray(Wq, dtype=np.float32)
    Wk = np.asarray(Wk, dtype=np.float32)
    Wv = np.asarray(Wv, dtype=np.float32)
    Wp = np.asarray(Wp, dtype=np.float32)
    bp = np.asarray(bp, dtype=np.float32)

    nc = _get_nc()
    in_maps = []
    for c in range(8):
        b, g = c // 2, c % 2
        hs = slice(g * NH, (g + 1) * NH)
        in_maps.append({
            "xt": np.ascontiguousarray(x[b].T).astype(BF16NP),
            "wq": Wq[hs].transpose(1, 0, 2).reshape(C, E).astype(BF16NP),
            "wk": Wk[hs].transpose(1, 0, 2).reshape(C, E).astype(BF16NP),
            "wv": Wv[hs].transpose(1, 0, 2).reshape(C, E).astype(BF16NP),
            "wo": np.ascontiguousarray(Wp[:, g * E:(g + 1) * E].T).astype(BF16NP),
        })
    res = run_bass_kernel_spmd(nc, in_maps, core_ids=list(range(8)))
    _CACHE["last_result"] = res
    y = np.empty((B, T, C), dtype=np.float32)
    for b in range(B):
        y[b] = res.results[2 * b]["y"] + res.results[2 * b + 1]["y"] + bp
    return y


# revision 14
# speedup vs baseline: 2.1221x; 1.0492x over previous
"""Multi-head causal attention (B=4, T=2048, C=1024, H=16, D=64) on 8 TRN2 cores.

Sharding: core c = (batch b = c//2, head-group g = c%2 of 8 heads).
Per core (its batch, its 8 heads), all matmuls in bf16 with fp32 PSUM accum:
  QT/KT = W^T x^T              [E=512, T] head-major rows (bf16)
  V     = x Wv                 [T, E] natural orientation, augmented with a
                               64-wide block of ones columns per head
  per head, per 1024-wide query slab:
    scores^T = K_h Q_h^T       [s-chunk 128, t 1024] psum, causal-skipped
    P~ = exp(scores/sqrt(D))   bf16; diagonal chunks zeroed via affine_select
    O'^T = [V_h | 1s]^T P~     psum rows 0-63 = numerator, 64-127 = denominator
                               broadcast to 64 rows for free
    O^T = O'^T * recip(denom)  two DVE ops per 512-half
  y_part = O^T^T Wo_g          [T, C] partial
Host: y_b = y_part(g=0) + y_part(g=1) + bp.

Scheduling: attention is ACT(exp)-paced, so stage-2 (QKV projection) slabs 2-3
and the first half of the output projection are emitted as small "filler"
pieces interleaved between attention chunks to keep the PE busy while ACT
works. The AV matmul for chunk j is emitted after scores for chunk j+2 so the
PE never waits head-of-line on exp/select.
"""

from collections import deque

import ml_dtypes
import numpy as np

import concourse.bacc as bacc
import concourse.mybir as mybir
import concourse.tile as tile
from concourse.bass_utils import run_bass_kernel_spmd

B, T, C, H, D = 4, 2048, 1024, 16, 64
NH = 8                 # heads per core
E = NH * D             # 512 per-core head width
P = 128
KO = C // P            # 8 contraction chunks for QKV proj
ET = E // P            # 4 e-tiles
SW = 512               # psum-bank width / stage-2 t-slab width
NSLAB = T // SW        # 4
TSL = 1024             # attention t-slab width (2 psum banks)
NSC = T // P           # 16 s-chunks
LOOKAHEAD = 2          # AV emission lag (chunks)
F32 = mybir.dt.float32
BF16 = mybir.dt.bfloat16
EXP = mybir.ActivationFunctionType.Exp
GE = mybir.AluOpType.is_ge
SCALE = float(D) ** -0.5
BF16NP = ml_dtypes.bfloat16

_CACHE: dict = {}


def _build():
    nc = bacc.Bacc("TRN2", target_bir_lowering=False, debug=False)
    xt_d = nc.dram_tensor("xt", [C, T], BF16, kind="ExternalInput")
    wq_d = nc.dram_tensor("wq", [C, E], BF16, kind="ExternalInput")
    wk_d = nc.dram_tensor("wk", [C, E], BF16, kind="ExternalInput")
    wv_d = nc.dram_tensor("wv", [C, E], BF16, kind="ExternalInput")
    wo_d = nc.dram_tensor("wo", [E, C], BF16, kind="ExternalInput")
    y_d = nc.dram_tensor("y", [T, C], F32, kind="ExternalOutput")

    xt_v = xt_d.ap().rearrange("(ko p) t -> p ko t", p=P)
    wq_v = wq_d.ap().rearrange("(ko p) e -> p ko e", p=P)
    wk_v = wk_d.ap().rearrange("(ko p) e -> p ko e", p=P)
    wv_v = wv_d.ap().rearrange("(ko p) e -> p ko e", p=P)
    wo_v = wo_d.ap().rearrange("(ko p) j -> p ko j", p=P)
    y_v = y_d.ap()

    with tile.TileContext(nc) as tc:
        with (
            tc.tile_pool(name="qkv", bufs=1) as qkv_pool,
            tc.tile_pool(name="vsb", bufs=1) as v_pool,
            tc.tile_pool(name="otp", bufs=1) as ot_pool,
            tc.tile_pool(name="wgt", bufs=1) as w_pool,
            tc.tile_pool(name="xsl", bufs=2) as x_pool,
            tc.tile_pool(name="wop", bufs=1) as wo_pool,
            tc.tile_pool(name="ptl", bufs=5) as pt_pool,
            tc.tile_pool(name="rcs", bufs=4) as r_pool,
            tc.tile_pool(name="ysb", bufs=4) as y_pool,
            tc.tile_pool(name="psu", bufs=4, space="PSUM") as ps_u,
            tc.tile_pool(name="psw", bufs=2, space="PSUM") as ps_w,
        ):
            QT = qkv_pool.tile([P, ET, T], BF16)
            KT = qkv_pool.tile([P, ET, T], BF16)
            # V augmented: cols 0-63 = head data, 64-127 = ones
            Vsb = v_pool.tile([P, NSC, NH, P], BF16)
            OT = ot_pool.tile([P, ET, T], BF16)
            ones_t = v_pool.tile([P, 1], F32)
            nc.gpsimd.memset(ones_t[:], 1.0)
            nc.vector.tensor_copy(
                Vsb[:, :, :, D:P],
                ones_t[:, 0:1, None, None].to_broadcast((P, NSC, NH, D)))

            wq_s = w_pool.tile([P, KO, E], BF16)
            wk_s = w_pool.tile([P, KO, E], BF16)
            wv_s = w_pool.tile([P, KO, E], BF16)
            wo_s = wo_pool.tile([P, ET, C], BF16)

            # ---- stage-2 pieces: one slab = 1 DMA + 12 psum-fill pieces ----
            def stage2_pieces(sl):
                xs = x_pool.tile([P, KO, SW], BF16, tag="xs")

                def dma():
                    nc.sync.dma_start(xs[:], xt_v[:, :, sl * SW:(sl + 1) * SW])
                yield dma
                for et in range(ET):
                    def qk_fill(et=et, xs=xs, w=wq_s, dst=QT):
                        pq = ps_u.tile([P, SW], F32, tag="ps")
                        for ko in range(KO):
                            nc.tensor.matmul(
                                pq[:], lhsT=w[:, ko, et * P:(et + 1) * P],
                                rhs=xs[:, ko, :],
                                start=(ko == 0), stop=(ko == KO - 1))
                        nc.vector.tensor_copy(
                            dst[:, et, sl * SW:(sl + 1) * SW], pq[:])
                    yield qk_fill
                    def k_fill(et=et, xs=xs, w=wk_s, dst=KT):
                        pk = ps_u.tile([P, SW], F32, tag="ps")
                        for ko in range(KO):
                            nc.tensor.matmul(
                                pk[:], lhsT=w[:, ko, et * P:(et + 1) * P],
                                rhs=xs[:, ko, :],
                                start=(ko == 0), stop=(ko == KO - 1))
                        nc.vector.tensor_copy(
                            dst[:, et, sl * SW:(sl + 1) * SW], pk[:])
                    yield k_fill
                for si in range(SW // P):
                    def v_fill(si=si, xs=xs):
                        pv = ps_u.tile([P, E], F32, tag="ps")
                        for ko in range(KO):
                            nc.tensor.matmul(
                                pv[:], lhsT=xs[:, ko, si * P:(si + 1) * P],
                                rhs=wv_s[:, ko, :],
                                start=(ko == 0), stop=(ko == KO - 1))
                        st = sl * (SW // P) + si
                        nc.vector.tensor_copy(
                            Vsb[:, st, :, 0:D],
                            pv[:].rearrange("p (h d) -> p h d", d=D))
                    yield v_fill

            # ---- projection pieces: one = 4 matmuls + copy + DMA out ----
            def proj_pieces(ttt_range):
                for ttt in ttt_range:
                    for jn in range(C // SW):
                        def piece(ttt=ttt, jn=jn):
                            py = ps_u.tile([P, SW], F32, tag="ps")
                            for ko in range(ET):
                                nc.tensor.matmul(
                                    py[:],
                                    lhsT=OT[:, ko, ttt * P:(ttt + 1) * P],
                                    rhs=wo_s[:, ko, jn * SW:(jn + 1) * SW],
                                    start=(ko == 0), stop=(ko == ET - 1))
                            ys = y_pool.tile([P, SW], F32, tag="ys")
                            nc.vector.tensor_copy(ys[:], py[:])
                            nc.sync.dma_start(
                                y_v[ttt * P:(ttt + 1) * P,
                                    jn * SW:(jn + 1) * SW], ys[:])
                        yield piece

            # ---- emit: slabs 0-1 up front (x slab DMA first, then weights
            # in first-use order so the first Q fill starts ASAP) ----
            gens = [stage2_pieces(0), stage2_pieces(1)]
            next(gens[0])()  # slab0 x DMA
            nc.gpsimd.dma_start(wq_s[:], wq_v)
            next(gens[1])()  # slab1 x DMA
            nc.gpsimd.dma_start(wk_s[:], wk_v)
            nc.gpsimd.dma_start(wv_s[:], wv_v)
            for g in gens:
                for piece in g:
                    piece()

            # ---- attention with filler interleaving ----
            av_q = deque()

            def flush(n_keep):
                while len(av_q) > n_keep:
                    emit, norm = av_q.popleft()
                    emit()
                    if norm is not None:
                        norm()

            def attention(blocks, fillers, fill_every):
                fillers = deque(fillers)
                job = 0
                for h, tt2 in blocks:
                    eth = h // 2
                    po = (h % 2) * D
                    t0 = tt2 * TSL
                    n_sc = (t0 + TSL) // P
                    last_lo = (t0 + SW) // P - 1
                    p_oA = ps_u.tile([P, SW], F32, tag="ps")
                    p_oB = ps_u.tile([P, SW], F32, tag="ps")
                    for sc in range(n_sc):
                        dlt = sc * P - t0
                        lo_valid = dlt < SW
                        lo = 0 if lo_valid else SW
                        p_w = ps_w.tile([P, TSL], F32, tag="pw")
                        kk = KT[po:po + D, eth, sc * P:(sc + 1) * P]
                        e0 = max(0, dlt)  # cols < dlt are never read
                        if lo_valid:
                            nc.tensor.matmul(
                                p_w[:, e0:SW], lhsT=kk,
                                rhs=QT[po:po + D, eth, t0 + e0:t0 + SW],
                                start=True, stop=True)
                            nc.tensor.matmul(
                                p_w[:, SW:TSL], lhsT=kk,
                                rhs=QT[po:po + D, eth, t0 + SW:t0 + TSL],
                                start=True, stop=True)
                        else:
                            nc.tensor.matmul(
                                p_w[:, e0:TSL], lhsT=kk,
                                rhs=QT[po:po + D, eth, t0 + e0:t0 + TSL],
                                start=True, stop=True)
                        p_t = pt_pool.tile([P, TSL], BF16, tag="pt")
                        nc.scalar.activation(
                            p_t[:, e0:TSL], p_w[:, e0:TSL], EXP, scale=SCALE)
                        if dlt >= 0:
                            w_hi = min(dlt + P, TSL)
                            nc.gpsimd.affine_select(
                                out=p_t[:, e0:w_hi], in_=p_t[:, e0:w_hi],
                                pattern=[[1, w_hi - e0]], compare_op=GE,
                                fill=0.0, base=e0 - dlt,
                                channel_multiplier=-1)

                        def mk_av(p_oA=p_oA, p_oB=p_oB, p_t=p_t, sc=sc, h=h,
                                  lo_valid=lo_valid, last_lo=last_lo,
                                  n_sc=n_sc, dlt=dlt):
                            def emit():
                                vv = Vsb[:, sc, h, :]
                                e0 = max(0, dlt) if sc > 0 else 0
                                if lo_valid:
                                    nc.tensor.matmul(
                                        p_oA[:, e0:SW], lhsT=vv,
                                        rhs=p_t[:, e0:SW],
                                        start=(sc == 0), stop=(sc == last_lo))
                                    nc.tensor.matmul(
                                        p_oB[:], lhsT=vv, rhs=p_t[:, SW:TSL],
                                        start=(sc == 0), stop=(sc == n_sc - 1))
                                else:
                                    nc.tensor.matmul(
                                        p_oB[:, e0 - SW:SW], lhsT=vv,
                                        rhs=p_t[:, e0:TSL],
                                        start=(sc == 0), stop=(sc == n_sc - 1))
                            return emit

                        def mk_norm(p_oA=p_oA, p_oB=p_oB, eth=eth, po=po,
                                    t0=t0):
                            def emit():
                                for half, p_o in ((0, p_oA), (1, p_oB)):
                                    ta = t0 + half * SW
                                    rcp = r_pool.tile([P, SW], F32, tag="rc")
                                    dsb = r_pool.tile([P, SW], F32, tag="db")
                                    nc.vector.tensor_copy(
                                        dsb[0:D, :], p_o[D:P, :])
                                    nc.vector.reciprocal_approx_fast(
                                        out=rcp[0:D, :], in_=dsb[0:D, :])
                                    nc.vector.tensor_mul(
                                        OT[po:po + D, eth, ta:ta + SW],
                                        p_o[0:D, :], rcp[0:D, :])
                            return emit

                        is_last = sc == n_sc - 1
                        av_q.append((mk_av(), mk_norm() if is_last else None))
                        flush(LOOKAHEAD)
                        job += 1
                        if fillers and job % fill_every == 0:
                            fillers.popleft()()
                while fillers:
                    fillers.popleft()()

            fill_b = list(stage2_pieces(2)) + list(stage2_pieces(3))
            attention([(h, 0) for h in range(NH)], fill_b, 2)

            def wo_dma():
                nc.sync.dma_start(wo_s[:], wo_v)
            fill_c = [wo_dma] + list(proj_pieces(range(0, T // P // 2)))
            attention([(h, 1) for h in range(NH)], fill_c, 7)
            flush(0)

            # ---- tail: remaining projection ----
            for piece in proj_pieces(range(T // P // 2, T // P)):
                piece()
    nc.compile()
    return nc


def _get_nc():
    if "nc" not in _CACHE:
        _CACHE["nc"] = _build()
    return _CACHE["nc"]


def kernel(x, Wq, Wk, Wv, Wp, bp):
    x = np.asarray(x, dtype=np.float32)
    Wq = np.asarray(Wq, dtype=np.float32)
    Wk = np.asarray(Wk, dtype=np.float32)
    Wv = np.asarray(Wv, dtype=np.float32)
    Wp = np.asarray(Wp, dtype=np.float32)
    bp = np.asarray(bp, dtype=np.float32)

    nc = _get_nc()
    in_maps = []
    for c in range(8):
        b, g = c // 2, c % 2
        hs = slice(g * NH, (g + 1) * NH)
        in_maps.append({
            "xt": np.ascontiguousarray(x[b].T).astype(BF16NP),
            "wq": Wq[hs].transpose(1, 0, 2).reshape(C, E).astype(BF16NP),
            "wk": Wk[hs].transpose(1, 0, 2).reshape(C, E).astype(BF16NP),
            "wv": Wv[hs].transpose(1, 0, 2).reshape(C, E).astype(BF16NP),
            "wo": np.ascontiguousarray(Wp[:, g * E:(g + 1) * E].T).astype(BF16NP),
        })
    res = run_bass_kernel_spmd(nc, in_maps, core_ids=list(range(8)))
    _CACHE["last_result"] = res
    y = np.empty((B, T, C), dtype=np.float32)
    for b in range(B):
        y[b] = res.results[2 * b]["y"] + res.results[2 * b + 1]["y"] + bp
    return y
